# revision 1
# baseline (speedup 1.0000x reference)
"""Trainium2 kernel for the algo/task performance-scan problem.

Restructuring: the lax.scan's only cross-step dependency is through the 64
scalars sig[:, lx[l]] read each step.  That scalar chain (O(A*L + L^2) work)
is computed on the host in float64.  Given the per-step coefficients
c[a,l] = eff[a] + s[a,l]*boost[a], the full field is a banded matmul

    result[a, l, t] = sum_{j<=l} mem[a]^(l-j) * c[a,j] * row_j[t]

(mem ~ 0.5, so terms with l-j > ~64 are below fp32 noise), followed by
sig = tanh(result / (2*diff))  (identity: 2*sigmoid(x)-1 = tanh(x/2)).

The device does the heavy part: per core, 8 algos x [512 l x 1024 t] field
as 64 matmuls [K=128, M=128, N=512] (float32r = full PE speed) + tanh on
the ACT engine + 16.8MB output DMA.  Sharding: 8 algos per core, 8 cores.
"""

import sys

sys.path.insert(0, "/opt/trn_rl_repo")

import numpy as np

A, T, L = 64, 1024, 512
NCORES = 8
ACORE = A // NCORES          # 8 algos per core
LT = 64                      # l-tile size
NLT = L // LT                # 8 l-tiles
NTB = T // 128               # 8 task blocks
NG = 2                       # act/psum groups per tb (4 l-tiles each)

_CACHE = {}


def _build_program():
    import concourse.tile as tile
    from concourse import bacc, mybir

    nc = bacc.Bacc("TRN2", target_bir_lowering=False, debug=False,
                   enable_asserts=False, num_devices=NCORES)
    f32 = mybir.dt.float32
    f32r = mybir.dt.float32r

    r_in = nc.dram_tensor("r", [L, T], f32r, kind="ExternalInput").ap()
    g_in = nc.dram_tensor("g", [NLT, 128, ACORE * LT], f32r,
                          kind="ExternalInput").ap()
    d_in = nc.dram_tensor("d", [128, NTB], f32, kind="ExternalInput").ap()
    out = nc.dram_tensor("out", [ACORE, T, L], f32,
                         kind="ExternalOutput").ap()

    # R chunk per l-tile: window j in [js, js+127], js = 0 if lt==0 else
    # 64*(lt-1).  Even-aligned windows (odd lt, and lt=0) come from "A"
    # chunks at j = 0,128,256,384; odd-aligned (even lt>=2) from "B"
    # chunks at j = 64,192,320.
    chunk_specs = [("rA0", 0), ("rA1", 128), ("rA2", 256), ("rA3", 384),
                   ("rB0", 64), ("rB1", 192), ("rB2", 320)]
    lt_chunk = ["rA0", "rA0", "rB0", "rA1", "rB1", "rA2", "rB2", "rA3"]

    with tile.TileContext(nc) as tc:
        with tc.tile_pool(name="consts", bufs=1) as consts, \
             tc.tile_pool(name="outp", bufs=2) as outp, \
             tc.tile_pool(name="ps", bufs=2, space="PSUM") as psp:

            dsc = consts.tile([128, NTB], f32, tag="dsc")
            nc.sync.dma_start(dsc[:], d_in[:])

            rt = {}
            for name, js in chunk_specs:
                t_ = consts.tile([128, T], f32r, tag=name)
                nc.sync.dma_start(t_[:], r_in[js:js + 128, :])
                rt[name] = t_

            gt = []
            for lt in range(NLT):
                t_ = consts.tile([128, ACORE * LT], f32r, tag=f"g{lt}")
                nc.sync.dma_start(t_[:], g_in[lt])
                gt.append(t_)

            for tb in range(NTB):
                out_sb = outp.tile([128, ACORE * L], f32, tag="osb")
                oview = out_sb[:].rearrange(
                    "p (a g s l) -> p g s a l", a=ACORE, g=NG, s=4, l=LT)
                for g in range(NG):
                    ps = psp.tile([128, 4 * 512], f32, tag="ps")
                    for sub in range(4):
                        lt = g * 4 + sub
                        nc.tensor.matmul(
                            ps[:, sub * 512:(sub + 1) * 512],
                            lhsT=rt[lt_chunk[lt]][:, tb * 128:(tb + 1) * 128],
                            rhs=gt[lt][:],
                            start=True, stop=True)
                    nc.scalar.activation(
                        oview[:, g],
                        ps[:].rearrange("p (s a l) -> p s a l", s=4, a=ACORE),
                        mybir.ActivationFunctionType.Tanh,
                        scale=dsc[:, tb:tb + 1])
                nc.sync.dma_start(
                    out.rearrange("a t l -> t a l")[tb * 128:(tb + 1) * 128],
                    out_sb[:].rearrange("p (a l) -> p a l", a=ACORE))

    nc.compile()
    return nc


def _host_chain(lx, task_matrix, task_difficulty, alg_efficiency,
                alg_memory, alg_experience_boost):
    """Exact (f64) scalar feedback chain + banded G coefficient tensors."""
    lx = np.asarray(lx).astype(np.int64)
    TM = np.asarray(task_matrix, dtype=np.float64)
    diff = np.asarray(task_difficulty, dtype=np.float64)
    eff = np.asarray(alg_efficiency, dtype=np.float64)
    mem = np.asarray(alg_memory, dtype=np.float64)
    boost = np.asarray(alg_experience_boost, dtype=np.float64)

    R = TM[lx]                     # [L, T]
    TM2 = R[:, lx]                 # [L, L]
    dlx = diff[lx]                 # [L]

    resS = np.zeros((A, L))
    c = np.empty((A, L))
    for l in range(L):
        s_l = 2.0 / (1.0 + np.exp(-resS[:, l] / dlx[l])) - 1.0
        c[:, l] = eff + s_l * boost
        resS = resS * mem[:, None] + c[:, l][:, None] * TM2[l][None, :]

    # G[a, lt, jj, ll] = mem^(l-j) * c[a, j], j = js(lt)+jj, l = 64*lt+ll
    pmat = mem[:, None] ** np.arange(192)[None, :]       # [A, 192]
    G = np.zeros((A, NLT, 128, LT), dtype=np.float64)
    for lt in range(NLT):
        js = 0 if lt == 0 else 64 * (lt - 1)
        jw = np.arange(js, js + 128)
        lmj = (np.arange(LT)[None, :] + 64 * lt) - jw[:, None]   # [128, LT]
        valid = lmj >= 0
        G[:, lt] = np.where(valid[None],
                            pmat[:, np.maximum(lmj, 0)] * c[:, jw][:, :, None],
                            0.0)
    # pack per core: [NLT, 128, ACORE*LT], free index a*LT+ll
    Gf = G.astype(np.float32)
    packs = []
    for core in range(NCORES):
        blk = Gf[core * ACORE:(core + 1) * ACORE]        # [ACORE, NLT, 128, LT]
        packs.append(np.ascontiguousarray(
            blk.transpose(1, 2, 0, 3).reshape(NLT, 128, ACORE * LT)))

    dsc = np.ascontiguousarray(
        (1.0 / (2.0 * diff)).reshape(NTB, 128).T).astype(np.float32)
    return R.astype(np.float32), packs, dsc


def kernel(lx, task_matrix, task_difficulty, alg_efficiency, alg_memory,
           alg_experience_boost):
    from concourse.bass_utils import run_bass_kernel_spmd

    R, packs, dsc = _host_chain(lx, task_matrix, task_difficulty,
                                alg_efficiency, alg_memory,
                                alg_experience_boost)

    if "nc" not in _CACHE:
        _CACHE["nc"] = _build_program()
    nc = _CACHE["nc"]

    in_maps = [{"r": R, "g": packs[c], "d": dsc} for c in range(NCORES)]
    res = run_bass_kernel_spmd(nc, in_maps, core_ids=list(range(NCORES)),
                               trace=False)
    field = np.concatenate([res.results[c]["out"] for c in range(NCORES)],
                           axis=0)                      # [A, T, L]
    out = np.empty((A, T, L + 1), dtype=np.float32)
    out[:, :, 0] = 0.0
    out[:, :, 1:] = field
    return out


# revision 3
# speedup vs baseline: 1.1310x; 1.1310x over previous
"""Trainium2 kernel for the algo/task performance-scan problem.

Restructuring: the lax.scan's only cross-step dependency is through the 64
scalars sig[:, lx[l]] read each step.  That scalar chain (O(A*L + L^2) work)
is computed on the host in float64.  Given the per-step coefficients
c[a,l] = eff[a] + s[a,l]*boost[a], the full field is a banded matmul

    result[a, l, t] = sum_{j<=l} mem[a]^(l-j) * c[a,j] * row_j[t]

(mem ~ 0.5-0.72, so terms with l-j > ~64 are below fp32 noise), followed by
sig = tanh(result / (2*diff))  (identity: 2*sigmoid(x)-1 = tanh(x/2)).

Device per core (8 algos): banded matmul as [K=128, M=128 t, N=512 (a,l)]
matmuls with an error-compensated bf16 split (R=Rh+Rl, G=Gh+Gl;
Rh@Gh + Rl@Gh + Rh@Gl accumulated in fp32 PSUM — ~2e-5 field error at
full bf16 PE speed), tanh on the ACT engine with per-partition 1/(2*diff)
scale, fp16 output DMA (host upcasts).  Sharding: 8 algos per core.
"""

import sys

sys.path.insert(0, "/opt/trn_rl_repo")

import numpy as np

A, T, L = 64, 1024, 512
NCORES = 8
ACORE = A // NCORES          # 8 algos per core
LT = 64                      # l-tile size
NLT = L // LT                # 8 l-tiles
NTB = T // 128               # 8 task blocks
NG = 2                       # psum groups per tb (4 l-tiles each)

_CACHE = {}


def _build_program():
    import concourse.tile as tile
    from concourse import bacc, mybir

    nc = bacc.Bacc("TRN2", target_bir_lowering=False, debug=False,
                   enable_asserts=False, num_devices=NCORES)
    f32 = mybir.dt.float32
    f16 = mybir.dt.float16
    bf16 = mybir.dt.bfloat16

    rh_in = nc.dram_tensor("rh", [L, T], bf16, kind="ExternalInput").ap()
    rl_in = nc.dram_tensor("rl", [L, T], bf16, kind="ExternalInput").ap()
    gh_in = nc.dram_tensor("gh", [NLT, 128, ACORE * LT], bf16,
                           kind="ExternalInput").ap()
    gl_in = nc.dram_tensor("gl", [NLT, 128, ACORE * LT], bf16,
                           kind="ExternalInput").ap()
    d_in = nc.dram_tensor("d", [128, NTB], f32, kind="ExternalInput").ap()
    out = nc.dram_tensor("out", [ACORE, T, L], f16,
                         kind="ExternalOutput").ap()

    # R chunk per l-tile: window j in [js, js+127], js = 0 if lt==0 else
    # 64*(lt-1).  Even-aligned windows (odd lt, and lt=0) come from "A"
    # chunks at j = 0,128,256,384; odd-aligned (even lt>=2) from "B"
    # chunks at j = 64,192,320.
    chunk_specs = [("A0", 0), ("A1", 128), ("A2", 256), ("A3", 384),
                   ("B0", 64), ("B1", 192), ("B2", 320)]
    lt_chunk = ["A0", "A0", "B0", "A1", "B1", "A2", "B2", "A3"]

    with tile.TileContext(nc) as tc:
        with tc.tile_pool(name="consts", bufs=1) as consts, \
             tc.tile_pool(name="outp", bufs=3) as outp, \
             tc.tile_pool(name="ps", bufs=2, space="PSUM") as psp:

            dsc = consts.tile([128, NTB], f32, tag="dsc")
            nc.sync.dma_start(dsc[:], d_in[:])

            rt = {}
            for name, js in chunk_specs:
                th = consts.tile([128, T], bf16, tag=f"rh{name}")
                nc.sync.dma_start(th[:], rh_in[js:js + 128, :])
                tl = consts.tile([128, T], bf16, tag=f"rl{name}")
                nc.sync.dma_start(tl[:], rl_in[js:js + 128, :])
                rt[name] = (th, tl)

            gt = []
            for lt in range(NLT):
                th = consts.tile([128, ACORE * LT], bf16, tag=f"gh{lt}")
                nc.sync.dma_start(th[:], gh_in[lt])
                tl = consts.tile([128, ACORE * LT], bf16, tag=f"gl{lt}")
                nc.sync.dma_start(tl[:], gl_in[lt])
                gt.append((th, tl))

            for tb in range(NTB):
                for g in range(NG):
                    ps = psp.tile([128, 4 * 512], f32, tag="ps")
                    for sub in range(4):
                        lt = g * 4 + sub
                        rh_t, rl_t = rt[lt_chunk[lt]]
                        gh_t, gl_t = gt[lt]
                        pslice = ps[:, sub * 512:(sub + 1) * 512]
                        lhs_h = rh_t[:, tb * 128:(tb + 1) * 128]
                        lhs_l = rl_t[:, tb * 128:(tb + 1) * 128]
                        nc.tensor.matmul(pslice, lhsT=lhs_h, rhs=gh_t[:],
                                         start=True, stop=False)
                        nc.tensor.matmul(pslice, lhsT=lhs_h, rhs=gl_t[:],
                                         start=False, stop=False)
                        nc.tensor.matmul(pslice, lhsT=lhs_l, rhs=gh_t[:],
                                         start=False, stop=True)
                    # out tile for (tb, g): free index a*256 + sub*64 + ll
                    osb = outp.tile([128, ACORE * 4 * LT], f16, tag="osb")
                    nc.scalar.activation(
                        osb[:].rearrange("p (a s l) -> p s a l", a=ACORE,
                                         s=4),
                        ps[:].rearrange("p (s a l) -> p s a l", s=4,
                                        a=ACORE),
                        mybir.ActivationFunctionType.Tanh,
                        scale=dsc[:, tb:tb + 1])
                    nc.sync.dma_start(
                        out.rearrange("a t l -> t a l")[
                            tb * 128:(tb + 1) * 128, :,
                            g * 256:(g + 1) * 256],
                        osb[:].rearrange("p (a l) -> p a l", a=ACORE))

    nc.compile()
    return nc


def _host_chain(lx, task_matrix, task_difficulty, alg_efficiency,
                alg_memory, alg_experience_boost):
    """Exact (f64) scalar feedback chain + banded G coefficient tensors."""
    import ml_dtypes
    bf = ml_dtypes.bfloat16

    lx = np.asarray(lx).astype(np.int64)
    TM = np.asarray(task_matrix, dtype=np.float64)
    diff = np.asarray(task_difficulty, dtype=np.float64)
    eff = np.asarray(alg_efficiency, dtype=np.float64)
    mem = np.asarray(alg_memory, dtype=np.float64)
    boost = np.asarray(alg_experience_boost, dtype=np.float64)

    R = TM[lx]                     # [L, T]
    TM2 = R[:, lx]                 # [L, L]
    dlx = diff[lx]                 # [L]

    resS = np.zeros((A, L))
    c = np.empty((A, L))
    for l in range(L):
        s_l = 2.0 / (1.0 + np.exp(-resS[:, l] / dlx[l])) - 1.0
        c[:, l] = eff + s_l * boost
        resS = resS * mem[:, None] + c[:, l][:, None] * TM2[l][None, :]

    Rf = R.astype(np.float32)
    Rh = Rf.astype(bf)
    Rl = (Rf - Rh.astype(np.float32)).astype(bf)

    # G[a, lt, jj, ll] = mem^(l-j) * c[a, j], j = js(lt)+jj, l = 64*lt+ll
    pmat = mem[:, None] ** np.arange(192)[None, :]       # [A, 192]
    G = np.zeros((A, NLT, 128, LT), dtype=np.float64)
    for lt in range(NLT):
        js = 0 if lt == 0 else 64 * (lt - 1)
        jw = np.arange(js, js + 128)
        lmj = (np.arange(LT)[None, :] + 64 * lt) - jw[:, None]   # [128, LT]
        valid = lmj >= 0
        G[:, lt] = np.where(valid[None],
                            pmat[:, np.maximum(lmj, 0)] * c[:, jw][:, :, None],
                            0.0)
    Gf = G.astype(np.float32)
    Gh = Gf.astype(bf)
    Gl = (Gf - Gh.astype(np.float32)).astype(bf)

    def pack(Gx):
        packs = []
        for core in range(NCORES):
            blk = Gx[core * ACORE:(core + 1) * ACORE]    # [ACORE,NLT,128,LT]
            packs.append(np.ascontiguousarray(
                blk.transpose(1, 2, 0, 3).reshape(NLT, 128, ACORE * LT)))
        return packs

    dsc = np.ascontiguousarray(
        (1.0 / (2.0 * diff)).reshape(NTB, 128).T).astype(np.float32)
    return Rh, Rl, pack(Gh), pack(Gl), dsc


def kernel(lx, task_matrix, task_difficulty, alg_efficiency, alg_memory,
           alg_experience_boost):
    from concourse.bass_utils import run_bass_kernel_spmd

    Rh, Rl, Ghp, Glp, dsc = _host_chain(lx, task_matrix, task_difficulty,
                                        alg_efficiency, alg_memory,
                                        alg_experience_boost)

    if "nc" not in _CACHE:
        _CACHE["nc"] = _build_program()
    nc = _CACHE["nc"]

    in_maps = [{"rh": Rh, "rl": Rl, "gh": Ghp[c], "gl": Glp[c], "d": dsc}
               for c in range(NCORES)]
    res = run_bass_kernel_spmd(nc, in_maps, core_ids=list(range(NCORES)),
                               trace=False)
    field = np.concatenate([res.results[c]["out"] for c in range(NCORES)],
                           axis=0)                      # [A, T, L] f16
    out = np.empty((A, T, L + 1), dtype=np.float32)
    out[:, :, 0] = 0.0
    out[:, :, 1:] = field.astype(np.float32)
    return out


# revision 4
# speedup vs baseline: 1.1826x; 1.0456x over previous
"""Trainium2 kernel for the algo/task performance-scan problem.

Restructuring: the lax.scan's only cross-step dependency is through the 64
scalars sig[:, lx[l]] read each step.  That scalar chain (O(A*L + L^2) work)
is computed on the host in float64.  Given the per-step coefficients
c[a,l] = eff[a] + s[a,l]*boost[a], the full field is a banded matmul

    result[a, l, t] = sum_{j<=l} mem[a]^(l-j) * c[a,j] * row_j[t]

(mem ~ 0.5-0.72, so terms with l-j > ~64 are below fp32 noise), followed by
sig = tanh(result / (2*diff))  (identity: 2*sigmoid(x)-1 = tanh(x/2)).

Device per core (8 algos): banded matmul as [K=128, M=128 t, N=512 (a,l)]
matmuls with an error-compensated bf16 split (R=Rh+Rl, G=Gh+Gl;
Rh@Gh + Rl@Gh + Rh@Gl accumulated in fp32 PSUM — ~2e-5 field error at
full bf16 PE speed), tanh on the ACT engine with per-partition 1/(2*diff)
scale, fp16 output DMA (host upcasts).  Sharding: 8 algos per core.
"""

import sys

sys.path.insert(0, "/opt/trn_rl_repo")

import numpy as np

A, T, L = 64, 1024, 512
NCORES = 8
ACORE = A // NCORES          # 8 algos per core
LT = 64                      # l-tile size
NLT = L // LT                # 8 l-tiles
NTB = T // 128               # 8 task blocks
NG = 2                       # psum groups per tb (4 l-tiles each)

_CACHE = {}


def _build_program():
    import concourse.tile as tile
    from concourse import bacc, mybir

    nc = bacc.Bacc("TRN2", target_bir_lowering=False, debug=False,
                   enable_asserts=False, num_devices=NCORES)
    f32 = mybir.dt.float32
    f16 = mybir.dt.float16
    bf16 = mybir.dt.bfloat16

    rh_in = nc.dram_tensor("rh", [L, T], bf16, kind="ExternalInput").ap()
    rl_in = nc.dram_tensor("rl", [L, T], bf16, kind="ExternalInput").ap()
    gh_in = nc.dram_tensor("gh", [NLT, 128, ACORE * LT], bf16,
                           kind="ExternalInput").ap()
    gl_in = nc.dram_tensor("gl", [NLT, 128, ACORE * LT], bf16,
                           kind="ExternalInput").ap()
    d_in = nc.dram_tensor("d", [128, NTB], f32, kind="ExternalInput").ap()
    out = nc.dram_tensor("out", [ACORE, T, L], f16,
                         kind="ExternalOutput").ap()

    # R chunk per l-tile: window j in [js, js+127], js = 0 if lt==0 else
    # 64*(lt-1).  Even-aligned windows (odd lt, and lt=0) come from "A"
    # chunks at j = 0,128,256,384; odd-aligned (even lt>=2) from "B"
    # chunks at j = 64,192,320.
    chunk_specs = [("A0", 0), ("A1", 128), ("A2", 256), ("A3", 384),
                   ("B0", 64), ("B1", 192), ("B2", 320)]
    lt_chunk = ["A0", "A0", "B0", "A1", "B1", "A2", "B2", "A3"]

    with tile.TileContext(nc) as tc:
        with tc.tile_pool(name="consts", bufs=1) as consts, \
             tc.tile_pool(name="outp", bufs=3) as outp, \
             tc.tile_pool(name="ps", bufs=2, space="PSUM") as psp:

            dsc = consts.tile([128, NTB], f32, tag="dsc")
            nc.sync.dma_start(dsc[:], d_in[:])

            # Emit loads in the order the g-outer compute loop consumes
            # them: the g=0 set (chunks A0,B0,A1 + G tiles 0-3) first so
            # the first PSUM group's operands land after ~2.8MB of DMA,
            # then the g=1 set streams in under g=0's compute.
            chunk_js = dict(chunk_specs)
            rt, gt = {}, {}

            def load_chunk(name):
                js = chunk_js[name]
                th = consts.tile([128, T], bf16, tag=f"rh{name}")
                nc.sync.dma_start(th[:], rh_in[js:js + 128, :])
                tl = consts.tile([128, T], bf16, tag=f"rl{name}")
                nc.sync.dma_start(tl[:], rl_in[js:js + 128, :])
                rt[name] = (th, tl)

            def load_g(lt):
                th = consts.tile([128, ACORE * LT], bf16, tag=f"gh{lt}")
                nc.sync.dma_start(th[:], gh_in[lt])
                tl = consts.tile([128, ACORE * LT], bf16, tag=f"gl{lt}")
                nc.sync.dma_start(tl[:], gl_in[lt])
                gt[lt] = (th, tl)

            for name, lts in [("A0", [0, 1]), ("B0", [2]), ("A1", [3]),
                              ("B1", [4]), ("A2", [5]), ("B2", [6]),
                              ("A3", [7])]:
                load_chunk(name)
                for lt in lts:
                    load_g(lt)

            for g in range(NG):
                for tb in range(NTB):
                    ps = psp.tile([128, 4 * 512], f32, tag="ps")
                    for sub in range(4):
                        lt = g * 4 + sub
                        rh_t, rl_t = rt[lt_chunk[lt]]
                        gh_t, gl_t = gt[lt]
                        pslice = ps[:, sub * 512:(sub + 1) * 512]
                        lhs_h = rh_t[:, tb * 128:(tb + 1) * 128]
                        lhs_l = rl_t[:, tb * 128:(tb + 1) * 128]
                        nc.tensor.matmul(pslice, lhsT=lhs_h, rhs=gh_t[:],
                                         start=True, stop=False)
                        nc.tensor.matmul(pslice, lhsT=lhs_h, rhs=gl_t[:],
                                         start=False, stop=False)
                        nc.tensor.matmul(pslice, lhsT=lhs_l, rhs=gh_t[:],
                                         start=False, stop=True)
                    # out tile for (tb, g): free index a*256 + sub*64 + ll
                    osb = outp.tile([128, ACORE * 4 * LT], f16, tag="osb")
                    nc.scalar.activation(
                        osb[:].rearrange("p (a s l) -> p s a l", a=ACORE,
                                         s=4),
                        ps[:].rearrange("p (s a l) -> p s a l", s=4,
                                        a=ACORE),
                        mybir.ActivationFunctionType.Tanh,
                        scale=dsc[:, tb:tb + 1])
                    nc.sync.dma_start(
                        out.rearrange("a t l -> t a l")[
                            tb * 128:(tb + 1) * 128, :,
                            g * 256:(g + 1) * 256],
                        osb[:].rearrange("p (a l) -> p a l", a=ACORE))

    nc.compile()
    return nc


def _host_chain(lx, task_matrix, task_difficulty, alg_efficiency,
                alg_memory, alg_experience_boost):
    """Exact (f64) scalar feedback chain + banded G coefficient tensors."""
    import ml_dtypes
    bf = ml_dtypes.bfloat16

    lx = np.asarray(lx).astype(np.int64)
    TM = np.asarray(task_matrix, dtype=np.float64)
    diff = np.asarray(task_difficulty, dtype=np.float64)
    eff = np.asarray(alg_efficiency, dtype=np.float64)
    mem = np.asarray(alg_memory, dtype=np.float64)
    boost = np.asarray(alg_experience_boost, dtype=np.float64)

    R = TM[lx]                     # [L, T]
    TM2 = R[:, lx]                 # [L, L]
    dlx = diff[lx]                 # [L]

    resS = np.zeros((A, L))
    c = np.empty((A, L))
    for l in range(L):
        s_l = 2.0 / (1.0 + np.exp(-resS[:, l] / dlx[l])) - 1.0
        c[:, l] = eff + s_l * boost
        resS = resS * mem[:, None] + c[:, l][:, None] * TM2[l][None, :]

    Rf = R.astype(np.float32)
    Rh = Rf.astype(bf)
    Rl = (Rf - Rh.astype(np.float32)).astype(bf)

    # G[a, lt, jj, ll] = mem^(l-j) * c[a, j], j = js(lt)+jj, l = 64*lt+ll
    pmat = mem[:, None] ** np.arange(192)[None, :]       # [A, 192]
    G = np.zeros((A, NLT, 128, LT), dtype=np.float64)
    for lt in range(NLT):
        js = 0 if lt == 0 else 64 * (lt - 1)
        jw = np.arange(js, js + 128)
        lmj = (np.arange(LT)[None, :] + 64 * lt) - jw[:, None]   # [128, LT]
        valid = lmj >= 0
        G[:, lt] = np.where(valid[None],
                            pmat[:, np.maximum(lmj, 0)] * c[:, jw][:, :, None],
                            0.0)
    Gf = G.astype(np.float32)
    Gh = Gf.astype(bf)
    Gl = (Gf - Gh.astype(np.float32)).astype(bf)

    def pack(Gx):
        packs = []
        for core in range(NCORES):
            blk = Gx[core * ACORE:(core + 1) * ACORE]    # [ACORE,NLT,128,LT]
            packs.append(np.ascontiguousarray(
                blk.transpose(1, 2, 0, 3).reshape(NLT, 128, ACORE * LT)))
        return packs

    dsc = np.ascontiguousarray(
        (1.0 / (2.0 * diff)).reshape(NTB, 128).T).astype(np.float32)
    return Rh, Rl, pack(Gh), pack(Gl), dsc


def kernel(lx, task_matrix, task_difficulty, alg_efficiency, alg_memory,
           alg_experience_boost):
    from concourse.bass_utils import run_bass_kernel_spmd

    Rh, Rl, Ghp, Glp, dsc = _host_chain(lx, task_matrix, task_difficulty,
                                        alg_efficiency, alg_memory,
                                        alg_experience_boost)

    if "nc" not in _CACHE:
        _CACHE["nc"] = _build_program()
    nc = _CACHE["nc"]

    in_maps = [{"rh": Rh, "rl": Rl, "gh": Ghp[c], "gl": Glp[c], "d": dsc}
               for c in range(NCORES)]
    res = run_bass_kernel_spmd(nc, in_maps, core_ids=list(range(NCORES)),
                               trace=False)
    field = np.concatenate([res.results[c]["out"] for c in range(NCORES)],
                           axis=0)                      # [A, T, L] f16
    out = np.empty((A, T, L + 1), dtype=np.float32)
    out[:, :, 0] = 0.0
    out[:, :, 1:] = field.astype(np.float32)
    return out


# revision 8
# speedup vs baseline: 1.2621x; 1.0672x over previous
"""Trainium2 kernel for the algo/task performance-scan problem.

Restructuring: the lax.scan's only cross-step dependency is through the 64
scalars sig[:, lx[l]] read each step.  That scalar chain (O(A*L + L^2) work)
is computed on the host in float64.  Given the per-step coefficients
c[a,l] = eff[a] + s[a,l]*boost[a], the full field is a banded matmul

    result[a, l, t] = sum_{j<=l} mem[a]^(l-j) * c[a,j] * row_j[t]

(mem ~ 0.5-0.72, so terms with l-j > ~64 are below fp32 noise), followed by
sig = tanh(result / (2*diff))  (identity: 2*sigmoid(x)-1 = tanh(x/2)).

Device per core (8 algos): banded matmul as [K=128, M=128 t, N=512 (a,l)]
matmuls with an error-compensated bf16 split (R=Rh+Rl, G=Gh+Gl;
Rh@Gh + Rl@Gh + Rh@Gl accumulated in fp32 PSUM — ~2e-5 field error at
full bf16 PE speed), tanh on the ACT engine with per-partition 1/(2*diff)
scale, fp16 output DMA (host upcasts).  Sharding: 8 algos per core.
"""

import sys

sys.path.insert(0, "/opt/trn_rl_repo")

import numpy as np

A, T, L = 64, 1024, 512
NCORES = 8
ACORE = A // NCORES          # 8 algos per core
LT = 64                      # l-tile size
NLT = L // LT                # 8 l-tiles
NTB = T // 128               # 8 task blocks
NG = 2                       # psum groups per tb (4 l-tiles each)

_CACHE = {}


def _build_program():
    import concourse.tile as tile
    from concourse import bacc, mybir

    nc = bacc.Bacc("TRN2", target_bir_lowering=False, debug=False,
                   enable_asserts=False, num_devices=NCORES)
    f32 = mybir.dt.float32
    f16 = mybir.dt.float16
    bf16 = mybir.dt.bfloat16

    rh_in = nc.dram_tensor("rh", [L, T], bf16, kind="ExternalInput").ap()
    rl_in = nc.dram_tensor("rl", [L, T], bf16, kind="ExternalInput").ap()
    gh_in = nc.dram_tensor("gh", [NLT, 128, ACORE * LT], bf16,
                           kind="ExternalInput").ap()
    gl_in = nc.dram_tensor("gl", [NLT, 128, ACORE * LT], bf16,
                           kind="ExternalInput").ap()
    d_in = nc.dram_tensor("d", [128, NTB], f32, kind="ExternalInput").ap()
    # [g, t, a, l-within-group] so each partition's store is one 4KB
    # contiguous run; the host permutes back to [a, t, l].
    out = nc.dram_tensor("out", [NG, T, ACORE, 256], f16,
                         kind="ExternalOutput").ap()

    # R chunk per l-tile: window j in [js, js+127], js = 0 if lt==0 else
    # 64*(lt-1).  Even-aligned windows (odd lt, and lt=0) come from "A"
    # chunks at j = 0,128,256,384; odd-aligned (even lt>=2) from "B"
    # chunks at j = 64,192,320.
    chunk_specs = [("A0", 0), ("A1", 128), ("A2", 256), ("A3", 384),
                   ("B0", 64), ("B1", 192), ("B2", 320)]
    lt_chunk = ["A0", "A0", "B0", "A1", "B1", "A2", "B2", "A3"]

    with tile.TileContext(nc) as tc:
        with tc.tile_pool(name="consts", bufs=1) as consts, \
             tc.tile_pool(name="outp", bufs=4) as outp, \
             tc.tile_pool(name="ps", bufs=2, space="PSUM") as psp:

            dsc = consts.tile([128, NTB], f32, tag="dsc")
            nc.sync.dma_start(dsc[:], d_in[:])

            # Emit loads in the order the g-outer compute loop consumes
            # them: the g=0 set (chunks A0,B0,A1 + G tiles 0-3) first so
            # the first PSUM group's operands land after ~2.8MB of DMA,
            # then the g=1 set streams in under g=0's compute.
            chunk_js = dict(chunk_specs)
            rt, gt = {}, {}

            def load_chunk(name):
                js = chunk_js[name]
                th = consts.tile([128, T], bf16, tag=f"rh{name}")
                nc.sync.dma_start(th[:], rh_in[js:js + 128, :])
                tl = consts.tile([128, T], bf16, tag=f"rl{name}")
                nc.sync.dma_start(tl[:], rl_in[js:js + 128, :])
                rt[name] = (th, tl)

            def load_g(lt):
                th = consts.tile([128, ACORE * LT], bf16, tag=f"gh{lt}")
                nc.sync.dma_start(th[:], gh_in[lt])
                tl = consts.tile([128, ACORE * LT], bf16, tag=f"gl{lt}")
                nc.sync.dma_start(tl[:], gl_in[lt])
                gt[lt] = (th, tl)

            for name, lts in [("A0", [0, 1]), ("B0", [2]), ("A1", [3]),
                              ("B1", [4]), ("A2", [5]), ("B2", [6]),
                              ("A3", [7])]:
                load_chunk(name)
                for lt in lts:
                    load_g(lt)

            for g in range(NG):
                for tb in range(NTB):
                    ps = psp.tile([128, 4 * 512], f32, tag="ps")
                    for sub in range(4):
                        lt = g * 4 + sub
                        rh_t, rl_t = rt[lt_chunk[lt]]
                        gh_t, gl_t = gt[lt]
                        pslice = ps[:, sub * 512:(sub + 1) * 512]
                        lhs_h = rh_t[:, tb * 128:(tb + 1) * 128]
                        lhs_l = rl_t[:, tb * 128:(tb + 1) * 128]
                        nc.tensor.matmul(pslice, lhsT=lhs_h, rhs=gh_t[:],
                                         start=True, stop=False)
                        nc.tensor.matmul(pslice, lhsT=lhs_h, rhs=gl_t[:],
                                         start=False, stop=False)
                        nc.tensor.matmul(pslice, lhsT=lhs_l, rhs=gh_t[:],
                                         start=False, stop=True)
                    # out tile for (tb, g): free index a*256 + sub*64 + ll
                    osb = outp.tile([128, ACORE * 4 * LT], f16, tag="osb")
                    nc.scalar.activation(
                        osb[:].rearrange("p (a s l) -> p s a l", a=ACORE,
                                         s=4),
                        ps[:].rearrange("p (s a l) -> p s a l", s=4,
                                        a=ACORE),
                        mybir.ActivationFunctionType.Tanh,
                        scale=dsc[:, tb:tb + 1])
                    nc.sync.dma_start(
                        out[g, tb * 128:(tb + 1) * 128],
                        osb[:].rearrange("p (a l) -> p a l", a=ACORE))

    nc.compile()
    return nc


def _host_chain(lx, task_matrix, task_difficulty, alg_efficiency,
                alg_memory, alg_experience_boost):
    """Exact (f64) scalar feedback chain + banded G coefficient tensors."""
    import ml_dtypes
    bf = ml_dtypes.bfloat16

    lx = np.asarray(lx).astype(np.int64)
    TM = np.asarray(task_matrix, dtype=np.float64)
    diff = np.asarray(task_difficulty, dtype=np.float64)
    eff = np.asarray(alg_efficiency, dtype=np.float64)
    mem = np.asarray(alg_memory, dtype=np.float64)
    boost = np.asarray(alg_experience_boost, dtype=np.float64)

    R = TM[lx]                     # [L, T]
    TM2 = R[:, lx]                 # [L, L]
    dlx = diff[lx]                 # [L]

    resS = np.zeros((A, L))
    c = np.empty((A, L))
    for l in range(L):
        s_l = 2.0 / (1.0 + np.exp(-resS[:, l] / dlx[l])) - 1.0
        c[:, l] = eff + s_l * boost
        resS = resS * mem[:, None] + c[:, l][:, None] * TM2[l][None, :]

    Rf = R.astype(np.float32)
    Rh = Rf.astype(bf)
    Rl = (Rf - Rh.astype(np.float32)).astype(bf)

    # G[a, lt, jj, ll] = mem^(l-j) * c[a, j], j = js(lt)+jj, l = 64*lt+ll
    pmat = mem[:, None] ** np.arange(192)[None, :]       # [A, 192]
    G = np.zeros((A, NLT, 128, LT), dtype=np.float64)
    for lt in range(NLT):
        js = 0 if lt == 0 else 64 * (lt - 1)
        jw = np.arange(js, js + 128)
        lmj = (np.arange(LT)[None, :] + 64 * lt) - jw[:, None]   # [128, LT]
        valid = lmj >= 0
        G[:, lt] = np.where(valid[None],
                            pmat[:, np.maximum(lmj, 0)] * c[:, jw][:, :, None],
                            0.0)
    Gf = G.astype(np.float32)
    Gh = Gf.astype(bf)
    Gl = (Gf - Gh.astype(np.float32)).astype(bf)

    def pack(Gx):
        packs = []
        for core in range(NCORES):
            blk = Gx[core * ACORE:(core + 1) * ACORE]    # [ACORE,NLT,128,LT]
            packs.append(np.ascontiguousarray(
                blk.transpose(1, 2, 0, 3).reshape(NLT, 128, ACORE * LT)))
        return packs

    dsc = np.ascontiguousarray(
        (1.0 / (2.0 * diff)).reshape(NTB, 128).T).astype(np.float32)
    return Rh, Rl, pack(Gh), pack(Gl), dsc


def kernel(lx, task_matrix, task_difficulty, alg_efficiency, alg_memory,
           alg_experience_boost):
    from concourse.bass_utils import run_bass_kernel_spmd

    Rh, Rl, Ghp, Glp, dsc = _host_chain(lx, task_matrix, task_difficulty,
                                        alg_efficiency, alg_memory,
                                        alg_experience_boost)

    if "nc" not in _CACHE:
        _CACHE["nc"] = _build_program()
    nc = _CACHE["nc"]

    in_maps = [{"rh": Rh, "rl": Rl, "gh": Ghp[c], "gl": Glp[c], "d": dsc}
               for c in range(NCORES)]
    res = run_bass_kernel_spmd(nc, in_maps, core_ids=list(range(NCORES)),
                               trace=False)
    out = np.empty((A, T, L + 1), dtype=np.float32)
    out[:, :, 0] = 0.0
    for c in range(NCORES):
        dev = res.results[c]["out"]          # [NG, T, ACORE, 256] f16
        # -> [ACORE, T, NG*256] then upcast into the output slab
        out[c * ACORE:(c + 1) * ACORE, :, 1:] = (
            dev.transpose(2, 1, 0, 3).reshape(ACORE, T, L).astype(np.float32))
    return out
